# revision 1
# baseline (speedup 1.0000x reference)
"""CQT (constant-Q transform) kernel for Trainium2, 8 NeuronCores.

Math: out[b, c, t] = sum_l W[c, l] * x_pad[b, t*HOP + l]   (strided conv,
HOP=512, L=11339 taps, C=168 channels = 84 bins x re/im), then reshaped to
(B, 2, n_bins, T_out).

Strategy:
  - Data-parallel: shard B=32 across 8 cores (4 batches/core), weights
    replicated.
  - The conv is decomposed into 128-tap blocks: block i covers taps
    [128*i, 128*i+128).  For each block, out[c, t] += Wblk_i[:, c].T @
    X_i[:, t] is a matmul with K=128 on partitions.  The moving operand for
    block i=(4j+k) at output tile [t0, t0+NT) is a contiguous column slice
    of a host-pre-transposed view of x:  xt[r, k, u] = x_pad[512u+128k+r].
  - CQT kernels are ragged (bin k has ~11339*2^(-k/12) taps, centered), so
    the active channels of each block form a prefix; blocks run with
    M = (last nonzero channel + 1) only.  PSUM accumulates all blocks per
    output tile (bank0 = channels 0:128, bank1 = 128:C), fp32r matmuls
    (1 cycle/row at N>=256, FP22 multiply / FP32 accumulate).
"""

import numpy as np

HOP = 512
N_CORES = 8

_prog_cache: dict = {}


def _host_prep(x, kernels):
    x = np.ascontiguousarray(np.asarray(x, dtype=np.float32))
    kernels = np.ascontiguousarray(np.asarray(kernels, dtype=np.float32))
    B, T = x.shape
    nbins, two, Lmax = kernels.shape
    assert two == 2
    C = 2 * nbins
    pad = Lmax // 2
    T_out = (T + 2 * pad - Lmax) // HOP + 1

    # ---- weights: pad taps to 128 multiple, find ragged active prefixes ----
    nblk_full = -(-Lmax // 128)
    Wp = np.zeros((C, nblk_full * 128), dtype=np.float32)
    Wp[:, :Lmax] = kernels.reshape(C, Lmax)
    nz = (Wp.reshape(C, nblk_full, 128) != 0.0).any(axis=2)  # [C, nblk]
    Ms, keep = [], []
    for i in range(nblk_full):
        idx = np.where(nz[:, i])[0]
        if len(idx):
            keep.append(i)
            Ms.append(int(idx[-1]) + 1)
    keep = np.asarray(keep, dtype=np.int64)
    Ms = np.asarray(Ms, dtype=np.int64)
    # order blocks by descending active-channel count: the first matmul per
    # PSUM bank then covers the bank's maximal partition range (required for
    # the start=True zero-region semantics), and the weight DMA can be
    # chunked in exactly the order the matmuls consume it.
    order = np.argsort(-Ms, kind="stable")
    keep = keep[order]
    Ms = Ms[order]
    # Ragged SBUF weight layout: block pos stores only its M_pos active
    # channels: wt[r, offs[pos] + c] = Wp[c, 128*keep[pos] + r], c < M_pos.
    # (4.4x less weight traffic than storing all C channels per block.)
    wblk = Wp.reshape(C, nblk_full, 128)
    wt = np.ascontiguousarray(
        np.concatenate(
            [wblk[:m, i, :].T for i, m in zip(keep, Ms)], axis=1
        )
    )
    offs = np.concatenate([[0], np.cumsum(Ms)]).tolist()
    keep = keep.tolist()
    Ms = Ms.tolist()

    # ---- x: pad and pre-transpose to [128, 4, U] per batch ----
    j_max = int(max(keep)) // 4
    U = T_out + j_max
    xpad_len = 512 * U
    assert xpad_len >= pad + T, (xpad_len, pad + T)
    xp = np.zeros((B, xpad_len), dtype=np.float32)
    xp[:, pad:pad + T] = x
    # xt[b, r, k*U + u] = xp[b, 512u + 128k + r]
    xt = np.ascontiguousarray(
        xp.reshape(B, U, 4, 128).transpose(0, 3, 2, 1).reshape(B, 128, 4 * U)
    )
    return xt, wt, keep, Ms, offs, C, U, T_out, nbins


def _build_program(b_per, C, U, T_out, keep, Ms, offs):
    import concourse.mybir as mybir
    import concourse.tile as tile
    from concourse import bacc

    f32 = mybir.dt.float32
    f32r = mybir.dt.float32r
    nblk = len(keep)
    sum_m = offs[-1]
    mb_max = max(max(Ms) - 128, 0)
    nts = [512] * (T_out // 512) + ([T_out % 512] if T_out % 512 else [])
    # blocks already ordered by descending M in host prep
    a_ps = list(range(nblk))
    b_ps = [p for p in a_ps if Ms[p] > 128]
    j_max = max(keep) // 4
    # weight DMA chunks in matmul consumption order; first chunks small so
    # the first matmuls' dependencies land as early as possible
    w_budgets = [192, 256, 512] + [704] * nblk
    w_chunks = []
    p0 = 0
    while p0 < nblk:
        budget = w_budgets[len(w_chunks)]
        p1 = p0 + 1
        while p1 < nblk and offs[p1 + 1] - offs[p0] <= budget:
            p1 += 1
        w_chunks.append((p0, p1))
        p0 = p1
    # x DMA chunks: one per t-tile window (u-ranges, exclusive ends)
    x_stops = []
    t0 = 0
    for nt in nts:
        x_stops.append(min(t0 + nt + j_max + 1, U))
        t0 += nt
    x_stops[-1] = U
    x_chunks = []
    u0 = 0
    for u1 in x_stops:
        if u1 > u0:
            x_chunks.append((u0, u1))
            u0 = u1

    nc = bacc.Bacc(
        "TRN2",
        target_bir_lowering=False,
        debug=False,
        enable_asserts=True,
        num_devices=N_CORES,
    )
    xt_d = nc.dram_tensor("xt", [b_per, 128, 4 * U], f32r, kind="ExternalInput").ap()
    wt_d = nc.dram_tensor("wt", [128, sum_m], f32r, kind="ExternalInput").ap()
    out_d = nc.dram_tensor("out", [b_per, C, T_out], f32, kind="ExternalOutput").ap()

    with tile.TileContext(nc) as tc:
        with (
            tc.tile_pool(name="wpool", bufs=1) as wpool,
            tc.tile_pool(name="xpool", bufs=2) as xpool,
            tc.tile_pool(name="evpool", bufs=3) as evpool,
            tc.tile_pool(name="pspool", bufs=2, space="PSUM") as pspool,
        ):
            wsb = wpool.tile([128, sum_m], f32r)

            def dma_x_chunk(xb_tile, b, u0, u1, ks):
                # 3D AP: k-planes ks (a contiguous range), u in [u0, u1)
                src = xt_d[b].rearrange("r (k u) -> r k u", k=4)
                dst = xb_tile.rearrange("r (k u) -> r k u", k=4)
                nc.sync.dma_start(
                    out=dst[:, ks[0]:ks[-1] + 1, u0:u1],
                    in_=src[:, ks[0]:ks[-1] + 1, u0:u1],
                )

            # interleave first batch's x chunks with the weight chunks (both
            # in consumption order) so the first sweep's matmuls start after
            # ~1MB of DMA instead of ~10MB.  The very first x window is
            # split per k-plane in first-use order.
            xb0 = xpool.tile([128, 4 * U], f32r, tag="xb", name="xb0")
            k_first = []
            for p in a_ps:
                k = keep[p] % 4
                if k not in k_first:
                    k_first.append(k)
            x_emits = [(x_chunks[0], (k,)) for k in k_first]
            x_emits += [(ch, (0, 1, 2, 3)) for ch in x_chunks[1:]]
            emits = []
            for i in range(max(len(x_emits), len(w_chunks))):
                if i < len(x_emits):
                    emits.append(("x", x_emits[i]))
                if i < len(w_chunks):
                    emits.append(("w", w_chunks[i]))
            for kind, args in emits:
                if kind == "x":
                    (u0, u1), ks = args
                    dma_x_chunk(xb0, 0, u0, u1, ks)
                else:
                    a0, a1 = args
                    nc.sync.dma_start(
                        out=wsb[:, offs[a0]:offs[a1]],
                        in_=wt_d[:, offs[a0]:offs[a1]],
                    )

            for b in range(b_per):
                if b == 0:
                    xb = xb0
                else:
                    xb = xpool.tile([128, 4 * U], f32r, tag="xb", name=f"xb{b}")
                    nc.sync.dma_start(out=xb[:], in_=xt_d[b])
                t0 = 0
                for nt in nts:
                    pa = pspool.tile([128, 512], f32, tag="pa")
                    if mb_max:
                        pb = pspool.tile([128, 512], f32, tag="pb")
                    for pos, p in enumerate(a_ps):
                        m = Ms[p]
                        j, k = divmod(keep[p], 4)
                        rhs = xb[:, k * U + t0 + j: k * U + t0 + j + nt]
                        ma = min(m, 128)
                        nc.tensor.matmul(
                            pa[:ma, :nt],
                            lhsT=wsb[:, offs[p]: offs[p] + ma],
                            rhs=rhs,
                            start=(pos == 0),
                            stop=(pos == len(a_ps) - 1),
                        )
                        if m > 128:
                            nc.tensor.matmul(
                                pb[:m - 128, :nt],
                                lhsT=wsb[:, offs[p] + 128: offs[p] + m],
                                rhs=rhs,
                                start=(p == b_ps[0]),
                                stop=(p == b_ps[-1]),
                            )
                    ma1 = min(Ms[a_ps[0]], 128)
                    eva = evpool.tile([128, 512], f32, tag="eva")
                    nc.vector.tensor_copy(eva[:ma1, :nt], pa[:ma1, :nt])
                    nc.sync.dma_start(
                        out=out_d[b, 0:ma1, t0:t0 + nt], in_=eva[:ma1, :nt]
                    )
                    if mb_max:
                        evb = evpool.tile([128, 512], f32, tag="evb")
                        nc.vector.tensor_copy(evb[:mb_max, :nt], pb[:mb_max, :nt])
                        nc.sync.dma_start(
                            out=out_d[b, 128:128 + mb_max, t0:t0 + nt],
                            in_=evb[:mb_max, :nt],
                        )
                    t0 += nt
    nc.compile()
    return nc


def _ensure_trace_shims():
    """If run_bass_kernel_spmd is invoked with tracing enabled (e.g. via
    BASS_TRACE=1) it imports antenv.axon_hooks and uploads artifacts to a
    bucket; neither exists in a bare container.  Register a working NTFF
    hook (ctypes into the axon .so) and a no-op uploader so the trace path
    degrades gracefully instead of crashing."""
    import sys

    try:
        import antenv.axon_hooks  # noqa: F401
    except ImportError:
        import contextlib
        import ctypes
        import types

        hook = None
        try:
            lib = ctypes.CDLL("/opt/axon/libaxon_pjrt.so")
            if hasattr(lib, "axon_start_nrt_profile"):
                lib.axon_start_nrt_profile.argtypes = [
                    ctypes.POINTER(ctypes.c_int64),
                    ctypes.c_size_t,
                ]
                lib.axon_start_nrt_profile.restype = ctypes.c_int64
                lib.axon_stop_nrt_profile.argtypes = [ctypes.c_char_p]
                lib.axon_stop_nrt_profile.restype = ctypes.c_int64

                @contextlib.contextmanager
                def _hook(output_dir, device_ids):
                    import jax

                    jax.devices()
                    if device_ids:
                        ids = (ctypes.c_int64 * len(device_ids))(*device_ids)
                        rc = lib.axon_start_nrt_profile(ids, len(device_ids))
                    else:
                        rc = lib.axon_start_nrt_profile(None, 0)
                    if rc != 0:
                        raise RuntimeError(f"axon_start_nrt_profile rc={rc}")
                    try:
                        yield
                    finally:
                        lib.axon_stop_nrt_profile(str(output_dir).encode())

                hook = _hook
        except OSError:
            pass
        mod = types.ModuleType("antenv.axon_hooks")
        mod.get_axon_ntff_profile_hook = lambda: hook
        mod.set_axon_ntff_profile_hook = lambda h: None
        sys.modules["antenv.axon_hooks"] = mod

    try:
        import concourse.bass_utils as _bu

        _orig_upload = _bu.upload_artifacts

        def _safe_upload(tmpdir):
            try:
                return _orig_upload(tmpdir)
            except Exception:
                return "local://unavailable"

        if not getattr(_bu, "_safe_upload_installed", False):
            _bu.upload_artifacts = _safe_upload
            _bu._safe_upload_installed = True
    except Exception:
        pass


def kernel(x, kernels):
    _ensure_trace_shims()
    from concourse.bass_utils import run_bass_kernel_spmd

    xt, wt, keep, Ms, offs, C, U, T_out, nbins = _host_prep(x, kernels)
    B = xt.shape[0]
    assert B % N_CORES == 0
    b_per = B // N_CORES

    key = (b_per, C, U, T_out, tuple(keep), tuple(Ms))
    if key not in _prog_cache:
        _prog_cache[key] = _build_program(b_per, C, U, T_out, keep, Ms, offs)
    nc = _prog_cache[key]

    in_maps = [
        {"xt": xt[c * b_per:(c + 1) * b_per], "wt": wt} for c in range(N_CORES)
    ]
    res = run_bass_kernel_spmd(nc, in_maps, list(range(N_CORES)))
    parts = [res.results[c]["out"] for c in range(N_CORES)]
    out = np.concatenate(parts, axis=0)  # (B, C, T_out)
    return np.ascontiguousarray(
        out.reshape(B, nbins, 2, T_out).transpose(0, 2, 1, 3)
    )



# revision 5
# speedup vs baseline: 1.7495x; 1.7495x over previous
"""CQT (constant-Q transform) kernel for Trainium2, 8 NeuronCores.

Math: out[b, c, t] = sum_l W[c, l] * x_pad[b, t*HOP + l]   (strided conv,
HOP=512, L=11339 taps, C=168 channels = 84 bins x re/im), then reshaped to
(B, 2, n_bins, T_out).

Strategy:
  - Data-parallel: shard B=32 across 8 cores (4 batches/core), weights
    replicated.
  - The conv is decomposed into 128-tap blocks: block p covers taps
    [128p, 128p+128).  The moving operand for block p=(4j+k) at output
    tile [t0, t0+nt) is a contiguous column slice of a host-pre-transposed
    view of x:  xt[r, k, u] = x_pad[512u + 128k + r].
  - CQT kernels are ragged (bin k has ~11339*2^(-k/12) taps, centered), so
    most blocks touch only a few low-bin channels.  A plain matmul costs
    ~N streaming cycles regardless of how few of the 128 PE columns hold
    weights, so the dense-block formulation wastes most of the array.
  - Column tiling: channels are split into groups of 32 (16 bins).  Each
    (block, group) quantum is a K=128, M<=32, N=nt matmul placed on one of
    the four 32-column PE tile positions (tile_position=(0, 32*slot)).
    The 4 tile positions stream concurrently, quartering PE time.
    Quanta per t-tile per group: {89, 36, 15, 7, 3, 2} = 152 vs 92
    full-width matmuls for the dense-block formulation; packed on 4 slots
    the makespan is 114 passes/batch vs 276 -> ~2.4x less PE streaming.
  - Each (group, t-tile) job accumulates its blocks into its own PSUM bank
    (per-element has_written semantics: first write overwrites, later ones
    accumulate), then DVE-copies psum[32s:32s+m] -> SBUF and DMAs to out.
    Static balanced schedule: 4 slots x 114 passes per batch.
"""

import numpy as np

HOP = 512
N_CORES = 8

_prog_cache: dict = {}


def _host_prep(x, kernels):
    x = np.ascontiguousarray(np.asarray(x, dtype=np.float32))
    kernels = np.ascontiguousarray(np.asarray(kernels, dtype=np.float32))
    B, T = x.shape
    nbins, two, Lmax = kernels.shape
    assert two == 2
    C = 2 * nbins
    pad = Lmax // 2
    T_out = (T + 2 * pad - Lmax) // HOP + 1

    # ---- weights: pad taps to 128 multiple ----
    nblk = -(-Lmax // 128)
    Wp = np.zeros((C, nblk * 128), dtype=np.float32)
    Wp[:, :Lmax] = kernels.reshape(C, Lmax)
    nzb = (Wp.reshape(C, nblk, 128) != 0.0).any(axis=2)  # [C, nblk]

    # channel groups of 32 (16 bins); bins are sorted by descending filter
    # length, supports are nested, so a group's active blocks = union over
    # its channels = the blocks of its longest (first) channel.
    groups = []  # (c0, m, blocks)
    for c0 in range(0, C, 32):
        m = min(32, C - c0)
        blks = np.where(nzb[c0:c0 + m].any(axis=0))[0].tolist()
        groups.append((c0, m, blks))

    # Weight layout: per (group, block) a zero-padded [128 taps, m chans]
    # panel; panels of a group are consecutive in block order.
    wcols = []
    tot = 0
    for (c0, m, blks) in groups:
        wcols.append(tot)
        tot += m * len(blks)
    wt = np.zeros((128, tot), dtype=np.float32)
    for (c0, m, blks), w0 in zip(groups, wcols):
        for rel, p in enumerate(blks):
            wt[:, w0 + rel * m: w0 + (rel + 1) * m] = \
                Wp[c0:c0 + m, 128 * p: 128 * (p + 1)].T
    import ml_dtypes
    wt = np.ascontiguousarray(wt.astype(ml_dtypes.bfloat16))

    # ---- x: pad and pre-transpose to [128, 4, U] per batch ----
    j_max = (nblk - 1) // 4
    U = T_out + j_max
    xpad_len = 512 * U
    assert xpad_len >= pad + T, (xpad_len, pad + T)
    xp = np.zeros((B, xpad_len), dtype=np.float32)
    xp[:, pad:pad + T] = x
    # xt[b, r, k*U + u] = xp[b, 512u + 128k + r]
    import ml_dtypes
    xt = np.ascontiguousarray(
        xp.reshape(B, U, 4, 128).transpose(0, 3, 2, 1).reshape(B, 128, 4 * U)
        .astype(ml_dtypes.bfloat16)
    )
    return xt, wt, groups, wcols, C, U, T_out, nbins


def _build_schedule(groups, T_out):
    """Static balanced 4-slot schedule.  Jobs are (group g, t-tile tt);
    job (g, tt) = len(groups[g].blocks) passes.  Slot loads for the CQT
    shape: 114/114/114/114 per batch."""
    nts = []
    t0 = 0
    while t0 < T_out:
        nts.append((t0, min(512, T_out - t0)))
        t0 += 512
    assert len(nts) == 3 and len(groups) == 6
    SLOT_JOBS = [
        [(0, 0), (2, 2), (3, 2), (4, 2)],
        [(1, 0), (1, 1), (1, 2), (5, 0), (5, 1), (5, 2)],
        [(2, 0), (3, 0), (4, 0), (0, 1)],
        [(2, 1), (3, 1), (4, 1), (0, 2)],
    ]
    # coverage check
    seen = set()
    for sj in SLOT_JOBS:
        for g, tt in sj:
            assert (g, tt) not in seen
            seen.add((g, tt))
    assert seen == {(g, tt) for g in range(6) for tt in range(3)}

    # flatten to per-slot quanta
    slot_q = []
    for sj in SLOT_JOBS:
        qs = []
        for g, tt in sj:
            blks = groups[g][2]
            for rel, p in enumerate(blks):
                qs.append((g, tt, p, rel, rel == 0, rel == len(blks) - 1))
        slot_q.append(qs)
    return nts, slot_q


def _build_program(b_per, C, U, T_out, groups, wcols):
    import concourse.mybir as mybir
    import concourse.tile as tile
    from concourse import bacc

    f32 = mybir.dt.float32
    bf16 = mybir.dt.bfloat16
    wtot = wcols[-1] + groups[-1][1] * len(groups[-1][2])
    nts, slot_q = _build_schedule(groups, T_out)
    maxlen = max(len(q) for q in slot_q)

    nc = bacc.Bacc(
        "TRN2",
        target_bir_lowering=False,
        debug=False,
        enable_asserts=True,
        num_devices=N_CORES,
    )
    xt_d = nc.dram_tensor("xt", [b_per, 128, 4 * U], bf16, kind="ExternalInput").ap()
    wt_d = nc.dram_tensor("wt", [128, wtot], bf16, kind="ExternalInput").ap()
    out_d = nc.dram_tensor("out", [b_per, C, T_out], f32, kind="ExternalOutput").ap()

    # weight DMA chunks, in rough consumption order: the head of each
    # group's block list is needed immediately (all slots start streaming
    # at pass 0), the tails later.
    w_chunks = []  # (g, rel0, rel1)
    HEAD = 12
    for g, (c0, m, blks) in enumerate(groups):
        w_chunks.append((g, 0, min(HEAD, len(blks))))
    for g, (c0, m, blks) in enumerate(groups):
        if len(blks) > HEAD:
            w_chunks.append((g, HEAD, min(4 * HEAD, len(blks))))
    for g, (c0, m, blks) in enumerate(groups):
        if len(blks) > 4 * HEAD:
            w_chunks.append((g, 4 * HEAD, len(blks)))

    # x DMA chunks for batch 0 (u-ranges); later batches get one big DMA.
    j_max = (max(groups[0][2])) // 4
    x_stops = []
    t0 = 0
    for (t0_, nt_) in nts:
        x_stops.append(min(t0_ + nt_ + j_max + 1, U))
    x_stops[-1] = U

    with tile.TileContext(nc) as tc:
        with (
            tc.tile_pool(name="wpool", bufs=1) as wpool,
            tc.tile_pool(name="xpool", bufs=2) as xpool,
            tc.tile_pool(name="evpool", bufs=2) as evpool,
            tc.tile_pool(name="pspool", bufs=2, space="PSUM") as pspool,
        ):
            wsb = wpool.tile([128, wtot], bf16)

            # interleave batch-0 x chunks with weight chunks
            xb0 = xpool.tile([128, 4 * U], bf16, tag="xb", name="xb0")
            emits = []
            u0 = 0
            for i, u1 in enumerate(x_stops):
                emits.append(("x", (u0, u1)))
                u0 = u1
            wi = 0
            merged = []
            for i, e in enumerate(emits):
                merged.append(e)
                # a few weight chunks between x chunks
                n_w = 6 if i == 0 else 3
                for _ in range(n_w):
                    if wi < len(w_chunks):
                        merged.append(("w", w_chunks[wi]))
                        wi += 1
            while wi < len(w_chunks):
                merged.append(("w", w_chunks[wi]))
                wi += 1
            for kind, args in merged:
                if kind == "x":
                    u0, u1 = args
                    src = xt_d[0].rearrange("r (k u) -> r k u", k=4)
                    dst = xb0.rearrange("r (k u) -> r k u", k=4)
                    nc.sync.dma_start(
                        out=dst[:, :, u0:u1], in_=src[:, :, u0:u1]
                    )
                else:
                    g, r0, r1 = args
                    c0, m, blks = groups[g]
                    a0 = wcols[g] + r0 * m
                    a1 = wcols[g] + r1 * m
                    nc.sync.dma_start(out=wsb[:, a0:a1], in_=wt_d[:, a0:a1])

            for b in range(b_per):
                if b == 0:
                    xb = xb0
                else:
                    xb = xpool.tile([128, 4 * U], bf16, tag="xb", name=f"xb{b}")
                    nc.sync.dma_start(out=xb[:], in_=xt_d[b])

                cur_ps = [None] * 4
                for i in range(maxlen):
                    for s in range(4):
                        if i >= len(slot_q[s]):
                            continue
                        g, tt, p, rel, first, last = slot_q[s][i]
                        c0, m, blks = groups[g]
                        t0, nt = nts[tt]
                        if first:
                            cur_ps[s] = pspool.tile(
                                [128, 512], f32, tag=f"ps{s}", name=f"ps{s}_{b}_{g}_{tt}"
                            )
                        ps = cur_ps[s]
                        j, k = divmod(p, 4)
                        wc = wcols[g] + rel * m
                        nc.tensor.matmul(
                            ps[32 * s: 32 * s + m, :nt],
                            lhsT=wsb[:, wc: wc + m],
                            rhs=xb[:, k * U + t0 + j: k * U + t0 + j + nt],
                            start=first,
                            stop=last,
                            tile_position=(0, 32 * s),
                        )
                        if last:
                            ev = evpool.tile(
                                [128, 512], f32, tag=f"ev{s}", name=f"ev{s}_{b}_{g}_{tt}"
                            )
                            nc.vector.tensor_copy(
                                ev[32 * s: 32 * s + m, :nt],
                                ps[32 * s: 32 * s + m, :nt],
                            )
                            nc.sync.dma_start(
                                out=out_d[b, c0:c0 + m, t0:t0 + nt],
                                in_=ev[32 * s: 32 * s + m, :nt],
                            )
    nc.compile()
    return nc


def _ensure_trace_shims():
    """If run_bass_kernel_spmd is invoked with tracing enabled (e.g. via
    BASS_TRACE=1) it imports antenv.axon_hooks and uploads artifacts to a
    bucket; neither exists in a bare container.  Register a working NTFF
    hook (ctypes into the axon .so) and a no-op uploader so the trace path
    degrades gracefully instead of crashing."""
    import sys

    try:
        import antenv.axon_hooks  # noqa: F401
    except ImportError:
        import contextlib
        import ctypes
        import types

        hook = None
        try:
            lib = ctypes.CDLL("/opt/axon/libaxon_pjrt.so")
            if hasattr(lib, "axon_start_nrt_profile"):
                lib.axon_start_nrt_profile.argtypes = [
                    ctypes.POINTER(ctypes.c_int64),
                    ctypes.c_size_t,
                ]
                lib.axon_start_nrt_profile.restype = ctypes.c_int64
                lib.axon_stop_nrt_profile.argtypes = [ctypes.c_char_p]
                lib.axon_stop_nrt_profile.restype = ctypes.c_int64

                @contextlib.contextmanager
                def _hook(output_dir, device_ids):
                    import jax

                    jax.devices()
                    if device_ids:
                        ids = (ctypes.c_int64 * len(device_ids))(*device_ids)
                        rc = lib.axon_start_nrt_profile(ids, len(device_ids))
                    else:
                        rc = lib.axon_start_nrt_profile(None, 0)
                    if rc != 0:
                        raise RuntimeError(f"axon_start_nrt_profile rc={rc}")
                    try:
                        yield
                    finally:
                        lib.axon_stop_nrt_profile(str(output_dir).encode())

                hook = _hook
        except OSError:
            pass
        mod = types.ModuleType("antenv.axon_hooks")
        mod.get_axon_ntff_profile_hook = lambda: hook
        mod.set_axon_ntff_profile_hook = lambda h: None
        sys.modules["antenv.axon_hooks"] = mod

    try:
        import concourse.bass_utils as _bu

        _orig_upload = _bu.upload_artifacts

        def _safe_upload(tmpdir):
            try:
                return _orig_upload(tmpdir)
            except Exception:
                return "local://unavailable"

        if not getattr(_bu, "_safe_upload_installed", False):
            _bu.upload_artifacts = _safe_upload
            _bu._safe_upload_installed = True
    except Exception:
        pass


def kernel(x, kernels):
    _ensure_trace_shims()
    from concourse.bass_utils import run_bass_kernel_spmd

    xt, wt, groups, wcols, C, U, T_out, nbins = _host_prep(x, kernels)
    B = xt.shape[0]
    assert B % N_CORES == 0
    b_per = B // N_CORES

    key = (b_per, C, U, T_out, tuple((c0, m, tuple(b)) for c0, m, b in groups))
    if key not in _prog_cache:
        _prog_cache[key] = _build_program(b_per, C, U, T_out, groups, wcols)
    nc = _prog_cache[key]

    in_maps = [
        {"xt": xt[c * b_per:(c + 1) * b_per], "wt": wt} for c in range(N_CORES)
    ]
    res = run_bass_kernel_spmd(nc, in_maps, list(range(N_CORES)))
    parts = [res.results[c]["out"] for c in range(N_CORES)]
    out = np.concatenate(parts, axis=0)  # (B, C, T_out)
    return np.ascontiguousarray(
        out.reshape(B, nbins, 2, T_out).transpose(0, 2, 1, 3)
    )
